# revision 1
# baseline (speedup 1.0000x reference)
"""DiffPool GNN encoder on 8 Trainium2 NeuronCores.

Data-parallel over graphs: core d owns graphs [16d, 16d+16) = node rows
[1024d, 1024d+1024). Host builds each core's dense A^T slab (bf16 0/1),
the per-graph block-diagonal 64x64 A blocks, x^T, and pre-chunked bf16
weights. The device kernel computes, per core:

  level 0:  Y = x @ [pWh0|eWh0]            (row-major, lhsT = x^T)
            Mt = Y^T @ AT_slab             (feature-major aggregation)
            pool chain -> softmax S0 -> block-diag S_bd [1024,160]
            emb chain  -> Z [1024,300] row-major
            X1T = Z^T @ S_bd, A1 = S^T A S (+ transposed variant), both
            via block-diag matmuls
  level 1:  same, 16 graphs x 10 nodes
  level 2:  emb only (pool softmax over k=1 is identically 1), X3 = per
            graph column sum of Z2
  head:     out^T = lW2^T @ relu(lW1^T @ X3T + lb1) + lb2   [128, 16]

Host gathers the 8 [128,16] outputs into the [128,128] result.
"""

import numpy as np
import ml_dtypes

BF = ml_dtypes.bfloat16
F8 = ml_dtypes.float8_e4m3fn
N_CORES = 8
N_NODES = 8192
B = 128
GPC = 16          # graphs per core
R = 1024          # rows per core
N0 = 64           # nodes per graph at level 0
D = 300
K0, K1 = 10, 4
K1NODES = 10  # nodes per graph at level 1

_prog_cache = {}


def _patch_tile_drain():
    """This container's walrus rejects >2 sync waits on one instruction;
    split the Tile tail-drain waits across several drains."""
    import concourse.tile as tile_mod
    from concourse.vector_clock import ScopedClock, VectorClock

    if getattr(tile_mod.TileContext, "_drain_patched", False):
        return

    def _patched(self, tick_clock, wait_clock):
        gc = tick_clock.global_clock
        n = len(gc)
        for start in range(0, n, 2):
            partial = VectorClock(
                [gc[p] if start <= p < start + 2 else 0 for p in range(n)]
            )
            di = self.nc.sync.drain()
            wait_clock.add_sem_waits(di.ins, ScopedClock({None: partial}))
        self.nc.all_engine_barrier()
        assert self.sems is not None
        popped = self.nc._tile_sem_poison_stack.pop()
        assert popped is self._sem_poison
        self.nc.clear_and_free_semaphores(list(self.sems.allocated().values()))
        self.nc.all_engine_barrier()

    tile_mod.TileContext._drain_and_barrier = _patched
    tile_mod.TileContext._drain_patched = True


def _split_excess_waits(nc, max_waits=1):
    """walrus here rejects instructions with >2 sync waits. Move excess waits
    onto injected same-engine nops placed immediately before the instruction
    (engine queues execute in order, so this preserves semantics)."""
    import concourse.mybir as mybir

    blocks = nc.m.functions[0].blocks
    for b in blocks:
        idx = 0
        while idx < len(b.instructions):
            inst = b.instructions[idx]
            si = inst.sync_info
            lim = max_waits
            if si is None or not si.on_wait or len(si.on_wait) <= lim:
                idx += 1
                continue
            waits = list(si.on_wait)
            keep = waits[-lim:]
            rest = waits[:-lim]
            inst.sync_info = mybir.SyncInfo(
                on_wait=keep, on_update=list(si.on_update or []))
            nops = []
            for c0 in range(0, len(rest)):
                n = nc.engines[inst.engine].nop(nofuse=True)
                ni = n.ins
                ni.sync_info = mybir.SyncInfo(
                    on_wait=[rest[c0]], on_update=[])
                # remove from wherever the builder appended it
                for b2 in blocks:
                    for j in range(len(b2.instructions) - 1, -1, -1):
                        if b2.instructions[j] is ni:
                            b2.instructions.pop(j)
                            break
                nops.append(ni)
            for n_off, ni in enumerate(nops):
                b.instructions.insert(idx + n_off, ni)
            idx += len(nops) + 1


def _softmax_rowmajor(nc, pool, psum_l, out_sb, k):
    """Row-major softmax over free dim k. psum_l: [p, k] f32 logits;
    out_sb: [p, k] bf16 destination."""
    import concourse.mybir as mybir

    p = psum_l.shape[0]
    mx = pool.tile([p, 1], mybir.dt.float32, tag="smax_mx")
    sm = pool.tile([p, 1], mybir.dt.float32, tag="smax_sum")
    rc = pool.tile([p, 1], mybir.dt.float32, tag="smax_rcp")
    ex = pool.tile([p, k], mybir.dt.float32, tag="smax_exp")
    nc.vector.reduce_max(mx[:], psum_l[:], axis=mybir.AxisListType.X, negate=True)
    nc.scalar.activation(
        ex[:], psum_l[:], mybir.ActivationFunctionType.Exp,
        bias=mx[:], scale=1.0, accum_out=sm[:],
    )
    nc.vector.reciprocal(rc[:], sm[:])
    nc.vector.tensor_scalar_mul(out_sb[:], ex[:], rc[:])


def _build_program():
    import concourse.bass as bass
    import concourse.mybir as mybir
    import concourse.tile as tile

    _patch_tile_drain()
    f32 = mybir.dt.float32
    bf16 = mybir.dt.bfloat16
    fp8 = mybir.dt.float8e4

    nc = bass.Bass()

    # ---- DRAM inputs (per-core shards handed via in_maps) ----
    d_xT = nc.dram_tensor("xT", [8, 100, 3, 1024], bf16, kind="ExternalInput")
    d_at = nc.dram_tensor("at", [32, 128, 2, 1024], fp8, kind="ExternalInput")
    d_adiag = nc.dram_tensor("adiag", [128, 8, 128], bf16, kind="ExternalInput")
    d_adiagT = nc.dram_tensor("adiagT", [128, 8, 128], bf16, kind="ExternalInput")
    d_wcat0 = nc.dram_tensor("wcat0", [100, 3, 492], bf16, kind="ExternalInput")
    d_pWl0 = nc.dram_tensor("pWl0", [128, 2, 300], bf16, kind="ExternalInput")
    d_pWo0 = nc.dram_tensor("pWo0", [100, 3, K0], bf16, kind="ExternalInput")
    d_eWl0 = nc.dram_tensor("eWl0", [128, 3, 600], bf16, kind="ExternalInput")
    d_eWo0 = nc.dram_tensor("eWo0", [120, 5, 300], bf16, kind="ExternalInput")
    d_pWh1 = nc.dram_tensor("pWh1", [100, 3, 150], bf16, kind="ExternalInput")
    d_pWl1 = nc.dram_tensor("pWl1", [75, 2, 300], bf16, kind="ExternalInput")
    d_pWo1 = nc.dram_tensor("pWo1", [100, 3, K1], bf16, kind="ExternalInput")
    d_eWh1 = nc.dram_tensor("eWh1", [100, 3, 300], bf16, kind="ExternalInput")
    d_eWl1 = nc.dram_tensor("eWl1", [100, 3, 600], bf16, kind="ExternalInput")
    d_eWo1 = nc.dram_tensor("eWo1", [120, 5, 300], bf16, kind="ExternalInput")
    d_eWh2 = nc.dram_tensor("eWh2", [100, 3, 300], bf16, kind="ExternalInput")
    d_eWl2 = nc.dram_tensor("eWl2", [100, 3, 600], bf16, kind="ExternalInput")
    d_eWo2 = nc.dram_tensor("eWo2", [120, 5, 300], bf16, kind="ExternalInput")
    d_lW1 = nc.dram_tensor("lW1", [100, 3, 600], bf16, kind="ExternalInput")
    d_lW2 = nc.dram_tensor("lW2", [120, 5, 128], bf16, kind="ExternalInput")
    d_lb1 = nc.dram_tensor("lb1", [120, 5], f32, kind="ExternalInput")
    d_lb2 = nc.dram_tensor("lb2", [128, 1], f32, kind="ExternalInput")
    d_ones = nc.dram_tensor("ones16", [64, GPC], bf16, kind="ExternalInput")
    d_s1mask = nc.dram_tensor("s1mask", [80, 2, 64], bf16, kind="ExternalInput")
    d_out = nc.dram_tensor("out", [128, GPC], f32, kind="ExternalOutput")

    with tile.TileContext(nc) as tc:
        with (
            tc.tile_pool(name="wpool", bufs=1) as wp,      # resident weights
            tc.tile_pool(name="big", bufs=1) as bigp,      # resident activations
            tc.tile_pool(name="atp", bufs=6) as atp,       # streamed AT tiles
            tc.tile_pool(name="tmp", bufs=4) as tmp,       # small temporaries
            tc.tile_pool(name="ps", bufs=8, space="PSUM") as psC,
        ):
            def load(dram, shape, eng=None):
                t = wp.tile(shape, dram.dtype, tag=dram.name)
                (eng or nc.scalar).dma_start(t[:], dram[:])
                return t

            wcat0 = load(d_wcat0, [100, 3, 492])
            xT = wp.tile([100, 3, N_NODES], bf16, tag="xT")
            for nq in range(8):
                nc.sync.dma_start(xT[:, :, nq * 1024:(nq + 1) * 1024], d_xT[nq])
            pWl0 = load(d_pWl0, [128, 2, 300])
            pWo0 = load(d_pWo0, [100, 3, K0])
            eWl0 = load(d_eWl0, [128, 3, 600])
            eWo0 = load(d_eWo0, [120, 5, 300])
            adiag = load(d_adiag, [128, 8, 128])
            adiagT = load(d_adiagT, [128, 8, 128])
            pWh1 = load(d_pWh1, [100, 3, 150])
            pWl1 = load(d_pWl1, [75, 2, 300])
            pWo1 = load(d_pWo1, [100, 3, K1])
            eWh1 = load(d_eWh1, [100, 3, 300])
            eWl1 = load(d_eWl1, [100, 3, 600])
            eWo1 = load(d_eWo1, [120, 5, 300])
            eWh2 = load(d_eWh2, [100, 3, 300])
            eWl2 = load(d_eWl2, [100, 3, 600])
            eWo2 = load(d_eWo2, [120, 5, 300])
            lW1 = load(d_lW1, [100, 3, 600])
            lW2 = load(d_lW2, [120, 5, 128])
            lb1 = load(d_lb1, [120, 5])
            lb2 = load(d_lb2, [128, 1])
            ones16 = load(d_ones, [64, GPC])

            Relu = mybir.ActivationFunctionType.Relu


            # ---- stage A: Y[1024*8? no: 8192, 450] row-major, bf16 ----
            # Y[128m+p, f] = sum_d x[128m+p, d] * wcat[d, f]
            Y = bigp.tile([128, 64, 512], fp8, tag="Y")
            for m in range(64):
                ps = psC.tile([128, 492], f32, tag="ps", name="psY")
                for kc in range(3):
                    nc.tensor.matmul(
                        ps[:], xT[:, kc, m * 128:(m + 1) * 128],
                        wcat0[:, kc, :],
                        start=(kc == 0), stop=(kc == 2),
                    )
                nc.vector.tensor_copy(Y[:, m, 0:492], ps[:])

            # ---- stage B: Mt = Y^T @ AT  (feature-major), relu -> G ----
            # feat chunks: pool [0:75),[75:150)  emb [150:250),[250:350),[350:450)
            Gp0 = bigp.tile([128, 1024], bf16, tag="Gp0")
            Gp1 = bigp.tile([32, 1024], bf16, tag="Gp1")
            Ge0 = bigp.tile([64, 1024], bf16, tag="Ge0")
            Ge1 = bigp.tile([128, 1024], bf16, tag="Ge1")
            Ge2 = bigp.tile([108, 1024], bf16, tag="Ge2")
            mchunks = [(0, 128), (128, 128), (256, 128), (384, 108)]
            pss = [[psC.tile([mchunks[mi][1], 512], f32, tag="ps",
                            name=f"psB_{nb}_{mi}")
                    for mi in range(4)] for nb in range(2)]
            for kk in range(0, 64, 2):
                at_t = atp.tile([128, 2, 1024], fp8, tag="at")
                nc.sync.dma_start(at_t[:], d_at[kk // 2])
                for nb in range(2):
                    for mi, (off, sz) in enumerate(mchunks):
                        nc.tensor.matmul(
                            pss[nb][mi][:],
                            Y[:, kk:kk + 2, off:off + sz],
                            at_t[:, :, nb * 512:(nb + 1) * 512],
                            start=(kk == 0), stop=(kk == 62),
                            perf_mode=mybir.MatmulPerfMode.DoubleRow,
                        )
            for nb in range(2):
                nbs = slice(nb * 512, (nb + 1) * 512)
                nc.scalar.activation(Gp0[:, nbs], pss[nb][0][:], Relu)
                nc.scalar.activation(Gp1[:, nbs], pss[nb][1][0:32, :], Relu)
                nc.scalar.activation(Ge0[:, nbs], pss[nb][1][64:128, :], Relu)
                nc.scalar.activation(Ge1[:, nbs], pss[nb][2][:], Relu)
                nc.scalar.activation(Ge2[:, nbs], pss[nb][3][:], Relu)

            # ---- level-0 chains (pool/emb interleaved for PE density) ----
            H1p = bigp.tile([100, 3, 1024], bf16, tag="H1p")
            H1e = bigp.tile([120, 5, 1024], bf16, tag="H1e")
            S_bd = bigp.tile([128, 8, 160], bf16, tag="S_bd")
            nc.any.memzero(S_bd[:])
            for nb in range(2):
                nbs = slice(nb * 512, (nb + 1) * 512)
                for mc in range(5):
                    ps = psC.tile([120, 512], f32, tag="ps", name="psH1e")
                    nc.tensor.matmul(
                        ps[:], eWl0[0:64, 0, mc * 120:(mc + 1) * 120],
                        Ge0[:, nbs], start=True, stop=False)
                    nc.tensor.matmul(
                        ps[:], eWl0[:, 1, mc * 120:(mc + 1) * 120],
                        Ge1[:, nbs], start=False, stop=False)
                    nc.tensor.matmul(
                        ps[:], eWl0[0:108, 2, mc * 120:(mc + 1) * 120],
                        Ge2[:, nbs], start=False, stop=True)
                    nc.scalar.activation(H1e[:, mc, nbs], ps[:], Relu)
                    if mc < 3:
                        ps2 = psC.tile([100, 512], f32, tag="ps", name="psH1p")
                        nc.tensor.matmul(
                            ps2[:], pWl0[:, 0, mc * 100:(mc + 1) * 100],
                            Gp0[:, nbs], start=True, stop=False)
                        nc.tensor.matmul(
                            ps2[:], pWl0[0:32, 1, mc * 100:(mc + 1) * 100],
                            Gp1[:, nbs], start=False, stop=True)
                        nc.scalar.activation(H1p[:, mc, nbs], ps2[:], Relu)

            # logits+softmax interleaved with Z
            Z = bigp.tile([128, 8, 300], bf16, tag="Z")
            for m in range(8):
                ps = psC.tile([128, K0], f32, tag="ps", name="psL")
                for kc in range(3):
                    nc.tensor.matmul(
                        ps[:], H1p[:, kc, m * 128:(m + 1) * 128], pWo0[:, kc, :],
                        start=(kc == 0), stop=(kc == 2),
                    )
                psz = psC.tile([128, 300], f32, tag="ps", name="psZ")
                for kc in range(5):
                    nc.tensor.matmul(
                        psz[:], H1e[:, kc, m * 128:(m + 1) * 128], eWo0[:, kc, :],
                        start=(kc == 0), stop=(kc == 4),
                    )
                nc.vector.tensor_copy(Z[:, m, :], psz[:])
                s_sb = tmp.tile([128, K0], bf16, tag="s0")
                _softmax_rowmajor(nc, tmp, ps, s_sb, K0)
                nc.vector.tensor_copy(
                    S_bd[0:64, m, m * 20:m * 20 + 10], s_sb[0:64, :])
                nc.vector.tensor_copy(
                    S_bd[64:128, m, m * 20 + 10:m * 20 + 20], s_sb[64:128, :])

            # ---- level-0 pooling ----
            # X1T[300, 160] = Z^T @ S_bd
            X1T = bigp.tile([100, 3, 160], bf16, tag="X1T")
            for mc in range(3):
                ps = psC.tile([100, 160], f32, tag="ps", name="psX1T")
                for k in range(8):
                    nc.tensor.matmul(
                        ps[:], Z[:, k, mc * 100:(mc + 1) * 100], S_bd[:, k, :],
                        start=(k == 0), stop=(k == 7),
                    )
                nc.vector.tensor_copy(X1T[:, mc, :], ps[:])

            # T_bd = A0_bd @ S_bd ; T2_bd = A0_bd^T @ S_bd  (block diag)
            T_bd = bigp.tile([128, 8, 160], bf16, tag="T_bd")
            T2_bd = bigp.tile([128, 8, 160], bf16, tag="T2_bd")
            nc.any.memzero(T_bd[:])
            nc.any.memzero(T2_bd[:])
            for c in range(8):
                psT = psC.tile([128, 20], f32, tag="ps", name="psT")
                nc.tensor.matmul(psT[:], adiagT[:, c, :],
                                 S_bd[:, c, c * 20:c * 20 + 20],
                                 start=True, stop=True)
                nc.vector.tensor_copy(T_bd[:, c, c * 20:c * 20 + 20], psT[:])
                psT2 = psC.tile([128, 20], f32, tag="ps", name="psT2")
                nc.tensor.matmul(psT2[:], adiag[:, c, :],
                                 S_bd[:, c, c * 20:c * 20 + 20],
                                 start=True, stop=True)
                nc.vector.tensor_copy(T2_bd[:, c, c * 20:c * 20 + 20], psT2[:])

            # A1_bd = S_bd^T @ T_bd ; A1T_bd = S_bd^T @ T2_bd   [160, 160]
            A1bd = bigp.tile([80, 2, 160], bf16, tag="A1bd")
            A1Tbd = bigp.tile([80, 2, 160], bf16, tag="A1Tbd")
            for mc in range(2):
                ps1 = psC.tile([80, 160], f32, tag="ps", name="psA1")
                ps2 = psC.tile([80, 160], f32, tag="ps", name="psA1T")
                for k in range(8):
                    nc.tensor.matmul(
                        ps1[:], S_bd[:, k, mc * 80:(mc + 1) * 80], T_bd[:, k, :],
                        start=(k == 0), stop=(k == 7))
                for k in range(8):
                    nc.tensor.matmul(
                        ps2[:], S_bd[:, k, mc * 80:(mc + 1) * 80], T2_bd[:, k, :],
                        start=(k == 0), stop=(k == 7))
                nc.vector.tensor_copy(A1bd[:, mc, :], ps1[:])
                nc.vector.tensor_copy(A1Tbd[:, mc, :], ps2[:])

            # ---- level 1 ----
            # Y1p [160, 150], Y1e [160, 300] row-major
            Y1p = bigp.tile([80, 2, 150], bf16, tag="Y1p")
            Y1e = bigp.tile([80, 2, 300], bf16, tag="Y1e")
            for mi in range(2):
                psp = psC.tile([80, 150], f32, tag="ps", name="psY1p")
                pse = psC.tile([80, 300], f32, tag="ps", name="psY1e")
                for kc in range(3):
                    nc.tensor.matmul(
                        psp[:], X1T[:, kc, mi * 80:(mi + 1) * 80], pWh1[:, kc, :],
                        start=(kc == 0), stop=(kc == 2))
                for kc in range(3):
                    nc.tensor.matmul(
                        pse[:], X1T[:, kc, mi * 80:(mi + 1) * 80], eWh1[:, kc, :],
                        start=(kc == 0), stop=(kc == 2))
                nc.vector.tensor_copy(Y1p[:, mi, :], psp[:])
                nc.vector.tensor_copy(Y1e[:, mi, :], pse[:])

            # M1pt [150, 160] = Y1p^T @ A1T_bd, relu -> G1p [75, 2, 160]
            G1p = bigp.tile([75, 2, 160], bf16, tag="G1p")
            for mf in range(2):
                ps = psC.tile([75, 160], f32, tag="ps", name="psM1p")
                for kc in range(2):
                    nc.tensor.matmul(
                        ps[:], Y1p[:, kc, mf * 75:(mf + 1) * 75], A1Tbd[:, kc, :],
                        start=(kc == 0), stop=(kc == 1))
                nc.scalar.activation(G1p[:, mf, :], ps[:], Relu)

            G1e = bigp.tile([100, 3, 160], bf16, tag="G1e")
            for mf in range(3):
                ps = psC.tile([100, 160], f32, tag="ps", name="psM1e")
                for kc in range(2):
                    nc.tensor.matmul(
                        ps[:], Y1e[:, kc, mf * 100:(mf + 1) * 100], A1Tbd[:, kc, :],
                        start=(kc == 0), stop=(kc == 1))
                nc.scalar.activation(G1e[:, mf, :], ps[:], Relu)

            # pool chain level 1
            H1p1 = bigp.tile([100, 3, 160], bf16, tag="H1p1")
            for mc in range(3):
                ps = psC.tile([100, 160], f32, tag="ps", name="psH1p1")
                for kc in range(2):
                    nc.tensor.matmul(
                        ps[:], pWl1[:, kc, mc * 100:(mc + 1) * 100], G1p[:, kc, :],
                        start=(kc == 0), stop=(kc == 1))
                nc.scalar.activation(H1p1[:, mc, :], ps[:], Relu)

            H1e1 = bigp.tile([120, 5, 160], bf16, tag="H1e1")
            for mc in range(5):
                ps = psC.tile([120, 160], f32, tag="ps", name="psH1e1")
                for kc in range(3):
                    nc.tensor.matmul(
                        ps[:], eWl1[:, kc, mc * 120:(mc + 1) * 120], G1e[:, kc, :],
                        start=(kc == 0), stop=(kc == 2))
                nc.scalar.activation(H1e1[:, mc, :], ps[:], Relu)

            S1_bd = bigp.tile([80, 2, 64], bf16, tag="S1_bd")
            s1mask = load(d_s1mask, [80, 2, 64])
            for mi in range(2):
                ps = psC.tile([80, K1], f32, tag="ps", name="psL1")
                for kc in range(3):
                    nc.tensor.matmul(
                        ps[:], H1p1[:, kc, mi * 80:(mi + 1) * 80], pWo1[:, kc, :],
                        start=(kc == 0), stop=(kc == 2))
                s_sb = tmp.tile([80, K1], bf16, tag="s1")
                _softmax_rowmajor(nc, tmp, ps, s_sb, K1)
                # block-diag scatter: replicate the [80,4] softmax 16x along
                # free dim and mask to the owning graph's 4 columns
                nc.vector.tensor_tensor(
                    S1_bd[:, mi, :].rearrange("p (b j) -> p b j", j=K1),
                    s_sb[:, None, :].to_broadcast((80, GPC, K1)),
                    s1mask[:, mi, :].rearrange("p (b j) -> p b j", j=K1),
                    mybir.AluOpType.mult)

            Z1 = bigp.tile([80, 2, 300], bf16, tag="Z1")
            for mi in range(2):
                ps = psC.tile([80, 300], f32, tag="ps", name="psZ1")
                for kc in range(5):
                    nc.tensor.matmul(
                        ps[:], H1e1[:, kc, mi * 80:(mi + 1) * 80], eWo1[:, kc, :],
                        start=(kc == 0), stop=(kc == 4))
                nc.vector.tensor_copy(Z1[:, mi, :], ps[:])

            # pooling level 1
            X2T = bigp.tile([100, 3, 64], bf16, tag="X2T")
            for mc in range(3):
                ps = psC.tile([100, 64], f32, tag="ps", name="psX2T")
                for kc in range(2):
                    nc.tensor.matmul(
                        ps[:], Z1[:, kc, mc * 100:(mc + 1) * 100], S1_bd[:, kc, :],
                        start=(kc == 0), stop=(kc == 1))
                nc.vector.tensor_copy(X2T[:, mc, :], ps[:])

            # T3 = A1_bd^T @ S1_bd ; A2T_bd = S1_bd^T @ T3   [64, 64]
            T3 = bigp.tile([80, 2, 64], bf16, tag="T3")
            for mi in range(2):
                ps = psC.tile([80, 64], f32, tag="ps", name="psT3")
                for kc in range(2):
                    nc.tensor.matmul(
                        ps[:], A1bd[:, kc, mi * 80:(mi + 1) * 80], S1_bd[:, kc, :],
                        start=(kc == 0), stop=(kc == 1))
                nc.vector.tensor_copy(T3[:, mi, :], ps[:])
            A2Tbd = bigp.tile([64, 64], bf16, tag="A2Tbd")
            psA2 = psC.tile([64, 64], f32, tag="ps", name="psA2T")
            for kc in range(2):
                nc.tensor.matmul(
                    psA2[:], S1_bd[:, kc, :], T3[:, kc, :],
                    start=(kc == 0), stop=(kc == 1))
            nc.vector.tensor_copy(A2Tbd[:], psA2[:])

            # ---- level 2 (emb only; S2 == 1) ----
            Y2 = bigp.tile([64, 300], bf16, tag="Y2")
            psY2 = psC.tile([64, 300], f32, tag="ps", name="psY2")
            for kc in range(3):
                nc.tensor.matmul(
                    psY2[:], X2T[:, kc, 0:64], eWh2[:, kc, :],
                    start=(kc == 0), stop=(kc == 2))
            nc.vector.tensor_copy(Y2[:], psY2[:])

            G2 = bigp.tile([100, 3, 64], bf16, tag="G2")
            for mf in range(3):
                ps = psC.tile([100, 64], f32, tag="ps", name="psM2")
                nc.tensor.matmul(
                    ps[:], Y2[:, mf * 100:(mf + 1) * 100], A2Tbd[:],
                    start=True, stop=True)
                nc.scalar.activation(G2[:, mf, :], ps[:], Relu)

            H2 = bigp.tile([120, 5, 64], bf16, tag="H2")
            for mc in range(5):
                ps = psC.tile([120, 64], f32, tag="ps", name="psH2")
                for kc in range(3):
                    nc.tensor.matmul(
                        ps[:], eWl2[:, kc, mc * 120:(mc + 1) * 120], G2[:, kc, :],
                        start=(kc == 0), stop=(kc == 2))
                nc.scalar.activation(H2[:, mc, :], ps[:], Relu)

            Z2 = bigp.tile([64, 300], bf16, tag="Z2")
            psZ2 = psC.tile([64, 300], f32, tag="ps", name="psZ2")
            for kc in range(5):
                nc.tensor.matmul(
                    psZ2[:], H2[:, kc, 0:64], eWo2[:, kc, :],
                    start=(kc == 0), stop=(kc == 4))
            nc.vector.tensor_copy(Z2[:], psZ2[:])

            # X3T [300, 16] = Z2^T @ ones_bd
            X3T = bigp.tile([100, 3, GPC], bf16, tag="X3T")
            for mf in range(3):
                ps = psC.tile([100, GPC], f32, tag="ps", name="psX3T")
                nc.tensor.matmul(
                    ps[:], Z2[:, mf * 100:(mf + 1) * 100], ones16[:],
                    start=True, stop=True)
                nc.vector.tensor_copy(X3T[:, mf, :], ps[:])

            # ---- head ----
            hT = bigp.tile([120, 5, GPC], bf16, tag="hT")
            for mc in range(5):
                ps = psC.tile([120, GPC], f32, tag="ps", name="psh")
                for kc in range(3):
                    nc.tensor.matmul(
                        ps[:], lW1[:, kc, mc * 120:(mc + 1) * 120], X3T[:, kc, :],
                        start=(kc == 0), stop=(kc == 2))
                nc.scalar.activation(hT[:, mc, :], ps[:], Relu,
                                     bias=lb1[:, mc:mc + 1])

            psO = psC.tile([128, GPC], f32, tag="ps", name="psO")
            for kc in range(5):
                nc.tensor.matmul(
                    psO[:], lW2[:, kc, :], hT[:, kc, :],
                    start=(kc == 0), stop=(kc == 4))
            outT = tmp.tile([128, GPC], f32, tag="outT")
            nc.vector.tensor_scalar_add(outT[:], psO[:], lb2[:])
            nc.sync.dma_start(d_out[:], outT[:])

    _split_excess_waits(nc)
    return nc


def _host_prep(inputs):
    """Build per-core in_maps from the full inputs."""
    ONE = np.uint8(0x38)  # 1.0 in float8_e4m3

    x = np.asarray(inputs["x"], np.float32)
    ei = np.asarray(inputs["edge_index"]).astype(np.int64)

    # full A^T in bf16 bit pattern: AT[j, i] = A[i, j]
    ATu = np.zeros((N_NODES, N_NODES), np.uint8)
    ATu[ei[1], ei[0]] = ONE

    xT = np.ascontiguousarray(
        x.T.reshape(3, 100, 8, 1024).transpose(2, 1, 0, 3)).astype(BF)

    def chunkw(w, p, c):
        w = np.asarray(w, np.float32)
        return np.ascontiguousarray(
            w.reshape(c, p, -1).transpose(1, 0, 2)).astype(BF)

    def padchunk(w, rowchunks, c, m):
        w = np.asarray(w, np.float32)
        out = np.zeros((128, c, m), np.float32)
        for ci, (a, b) in enumerate(rowchunks):
            out[0:b - a, ci, :] = w[a:b, :]
        return out.astype(BF)

    wcat0 = np.zeros((300, 492), np.float32)
    wcat0[:, 0:150] = np.asarray(inputs["pWh0"], np.float32)
    wcat0[:, 192:492] = np.asarray(inputs["eWh0"], np.float32)

    ones16 = np.zeros((64, GPC), BF)
    for b in range(GPC):
        ones16[b * 4:(b + 1) * 4, b] = 1
    s1mask = np.zeros((80, 2, 64), BF)
    for mi in range(2):
        for p in range(80):
            gb = (80 * mi + p) // K1NODES
            s1mask[p, mi, gb * 4:(gb + 1) * 4] = 1
    lb1 = np.ascontiguousarray(
        np.asarray(inputs["lb1"], np.float32).reshape(5, 120).T)
    lb2 = np.asarray(inputs["lb2"], np.float32).reshape(128, 1)

    shared = {
        "xT": xT,
        "wcat0": chunkw(wcat0, 100, 3),
        "pWl0": padchunk(inputs["pWl0"], [(0, 128), (128, 150)], 2, 300),
        "pWo0": chunkw(inputs["pWo0"], 100, 3),
        "eWl0": padchunk(inputs["eWl0"], [(0, 64), (64, 192), (192, 300)], 3, 600),
        "eWo0": chunkw(inputs["eWo0"], 120, 5),
        "pWh1": chunkw(inputs["pWh1"], 100, 3),
        "pWl1": chunkw(inputs["pWl1"], 75, 2),
        "pWo1": chunkw(inputs["pWo1"], 100, 3),
        "eWh1": chunkw(inputs["eWh1"], 100, 3),
        "eWl1": chunkw(inputs["eWl1"], 100, 3),
        "eWo1": chunkw(inputs["eWo1"], 120, 5),
        "eWh2": chunkw(inputs["eWh2"], 100, 3),
        "eWl2": chunkw(inputs["eWl2"], 100, 3),
        "eWo2": chunkw(inputs["eWo2"], 120, 5),
        "lW1": chunkw(inputs["lW1"], 100, 3),
        "lW2": chunkw(inputs["lW2"], 120, 5),
        "lb1": lb1,
        "lb2": lb2,
        "ones16": ones16,
        "s1mask": s1mask,
    }

    in_maps = []
    for d in range(N_CORES):
        r0 = d * R
        slab = ATu[:, r0:r0 + R]  # [8192, 1024]
        at = np.ascontiguousarray(
            slab.reshape(32, 2, 128, 1024).transpose(0, 2, 1, 3)).view(F8)

        adiag = np.zeros((128, 8, 128), np.uint8)
        adiagT = np.zeros((128, 8, 128), np.uint8)
        for c in range(8):
            # full 128x128 slab block, then mask to per-graph 64x64 diag
            blkT = slab[r0 + 128 * c: r0 + 128 * (c + 1),
                        128 * c: 128 * (c + 1)]  # blkT[q, p] = A[rows p, cols q]
            blk = blkT.T
            for h in range(2):
                s = slice(64 * h, 64 * (h + 1))
                adiag[s, c, s] = blk[s, s]
                adiagT[s, c, s] = blkT[s, s]
        m = dict(shared)
        m["at"] = at
        m["adiag"] = adiag.view(F8).astype(BF)
        m["adiagT"] = adiagT.view(F8).astype(BF)
        in_maps.append(m)
    return in_maps


def _run(inputs, trace=False, trace_kwargs=None):
    try:
        import concourse.bass as bass  # noqa: F401
    except ImportError:
        import sys
        sys.path.insert(0, "/opt/trn_rl_repo")
    from concourse.bass_utils import run_bass_kernel_spmd

    if "prog" not in _prog_cache:
        _prog_cache["prog"] = _build_program()
    nc = _prog_cache["prog"]

    in_maps = _host_prep(inputs)
    res = run_bass_kernel_spmd(
        nc, in_maps, core_ids=list(range(N_CORES)), trace=trace,
        **(trace_kwargs or {}),
    )
    out = np.empty((B, 128), np.float32)
    for d in range(N_CORES):
        out[d * GPC:(d + 1) * GPC, :] = res.results[d]["out"].T
    return out, res


def kernel(**inputs):
    out, _ = _run(inputs, trace=False)
    return out



# revision 12
# speedup vs baseline: 1.1963x; 1.1963x over previous
"""DiffPool GNN encoder on 8 Trainium2 NeuronCores.

Data-parallel over graphs: core d owns graphs [16d, 16d+16) = node rows
[1024d, 1024d+1024). Key restructuring vs the straightforward scheme:
relu(A @ (X @ Wh)) == relu((A @ X) @ Wh), so each core aggregates the
raw 300-wide X once (fp8 DoubleRow against its A^T slab) instead of the
492-wide X@[Wh_p|Wh_e] — this removes the duplicated X@W matmul on
every core and shrinks the big aggregation.

Pipeline per core (i = the core's 1024 node rows, split in halves h):

  B'(h):  Mxt[:, h] = x_f8^T @ AT_slab[:, h]    192 fp8 DR matmuls total
  A'(h):  G = relu(wcat0^T @ Mxt)               per-half, feature-major
  chain(h): pool/emb 2nd+3rd layers, softmax S0 (no max-subtract; logits
          are bounded ~5), Z, X1 = S^T Z, A1 = S^T A S  (block diag)
  L1(h)/L2(h): same per-graph pipeline on 10- then 4-node graphs
  head:   out^T = lW2^T @ relu(lW1^T @ X3T + lb1) + lb2   [128, 16]

Everything after the B' aggregation is per-graph, so all levels split
cleanly by half; emission interleaves half-1 tensor work into half-0's
vector/scalar latency (and B'(h1) into chain(h0)) to keep the PE fed.
Host gathers the 8 [128,16] outputs into the [128,128] result.
"""

import itertools

import numpy as np
import ml_dtypes

BF = ml_dtypes.bfloat16
F8 = ml_dtypes.float8_e4m3fn
N_CORES = 8
N_NODES = 8192
B = 128
GPC = 16          # graphs per core
R = 1024          # rows per core
D = 300
K0, K1 = 10, 4

_prog_cache = {}


def _patch_tile_drain():
    """This container's walrus rejects >2 sync waits on one instruction;
    split the Tile tail-drain waits across several drains."""
    import concourse.tile as tile_mod
    from concourse.vector_clock import ScopedClock, VectorClock

    if getattr(tile_mod.TileContext, "_drain_patched", False):
        return

    def _patched(self, tick_clock, wait_clock):
        gc = tick_clock.global_clock
        n = len(gc)
        for start in range(0, n, 2):
            partial = VectorClock(
                [gc[p] if start <= p < start + 2 else 0 for p in range(n)]
            )
            di = self.nc.sync.drain()
            wait_clock.add_sem_waits(di.ins, ScopedClock({None: partial}))
        self.nc.all_engine_barrier()
        assert self.sems is not None
        popped = self.nc._tile_sem_poison_stack.pop()
        assert popped is self._sem_poison
        self.nc.clear_and_free_semaphores(list(self.sems.allocated().values()))
        self.nc.all_engine_barrier()

    tile_mod.TileContext._drain_and_barrier = _patched
    tile_mod.TileContext._drain_patched = True


def _split_excess_waits(nc, max_waits=1):
    """walrus here rejects instructions with >2 sync waits. Move excess waits
    onto injected same-engine nops placed immediately before the instruction
    (engine queues execute in order, so this preserves semantics)."""
    import concourse.mybir as mybir

    blocks = nc.m.functions[0].blocks
    for b in blocks:
        idx = 0
        while idx < len(b.instructions):
            inst = b.instructions[idx]
            si = inst.sync_info
            lim = max_waits
            if si is None or not si.on_wait or len(si.on_wait) <= lim:
                idx += 1
                continue
            waits = list(si.on_wait)
            keep = waits[-lim:]
            rest = waits[:-lim]
            inst.sync_info = mybir.SyncInfo(
                on_wait=keep, on_update=list(si.on_update or []))
            nops = []
            for c0 in range(0, len(rest)):
                n = nc.engines[inst.engine].nop(nofuse=True)
                ni = n.ins
                ni.sync_info = mybir.SyncInfo(
                    on_wait=[rest[c0]], on_update=[])
                # remove from wherever the builder appended it
                for b2 in blocks:
                    for j in range(len(b2.instructions) - 1, -1, -1):
                        if b2.instructions[j] is ni:
                            b2.instructions.pop(j)
                            break
                nops.append(ni)
            for n_off, ni in enumerate(nops):
                b.instructions.insert(idx + n_off, ni)
            idx += len(nops) + 1


def _build_program():
    import concourse.bass as bass
    import concourse.mybir as mybir
    import concourse.tile as tile

    _patch_tile_drain()
    f32 = mybir.dt.float32
    bf16 = mybir.dt.bfloat16
    fp8 = mybir.dt.float8e4
    Relu = mybir.ActivationFunctionType.Relu
    Exp = mybir.ActivationFunctionType.Exp
    DR = mybir.MatmulPerfMode.DoubleRow
    AX = mybir.AxisListType.X
    Mul = mybir.AluOpType.mult

    nc = bass.Bass()

    # ---- DRAM inputs ----
    d_xf8 = nc.dram_tensor("xf8", [8, 128, 4, 2, 336], fp8, kind="ExternalInput")
    d_at = nc.dram_tensor("at", [64, 128, 2, 512], fp8, kind="ExternalInput")
    d_adiag = nc.dram_tensor("adiag", [128, 8, 128], bf16, kind="ExternalInput")
    d_adiagT = nc.dram_tensor("adiagT", [128, 8, 128], bf16, kind="ExternalInput")
    d_wcat0 = nc.dram_tensor("wcat0", [100, 3, 492], bf16, kind="ExternalInput")
    d_pWl0 = nc.dram_tensor("pWl0", [128, 2, 300], bf16, kind="ExternalInput")
    d_pWo0 = nc.dram_tensor("pWo0", [100, 3, K0], bf16, kind="ExternalInput")
    d_eWl0 = nc.dram_tensor("eWl0", [128, 3, 600], bf16, kind="ExternalInput")
    d_eWo0 = nc.dram_tensor("eWo0", [120, 5, 300], bf16, kind="ExternalInput")
    d_wcat1 = nc.dram_tensor("wcat1", [100, 3, 450], bf16, kind="ExternalInput")
    d_pWl1 = nc.dram_tensor("pWl1", [75, 2, 300], bf16, kind="ExternalInput")
    d_pWo1 = nc.dram_tensor("pWo1", [100, 3, K1], bf16, kind="ExternalInput")
    d_eWl1 = nc.dram_tensor("eWl1", [100, 3, 600], bf16, kind="ExternalInput")
    d_eWo1 = nc.dram_tensor("eWo1", [120, 5, 300], bf16, kind="ExternalInput")
    d_eWh2 = nc.dram_tensor("eWh2", [100, 3, 300], bf16, kind="ExternalInput")
    d_eWl2 = nc.dram_tensor("eWl2", [100, 3, 600], bf16, kind="ExternalInput")
    d_eWo2 = nc.dram_tensor("eWo2", [120, 5, 300], bf16, kind="ExternalInput")
    d_lW1 = nc.dram_tensor("lW1", [100, 3, 600], bf16, kind="ExternalInput")
    d_lW2 = nc.dram_tensor("lW2", [120, 5, 128], bf16, kind="ExternalInput")
    d_lb1 = nc.dram_tensor("lb1", [120, 5], f32, kind="ExternalInput")
    d_lb2 = nc.dram_tensor("lb2", [128, 1], f32, kind="ExternalInput")
    d_ones = nc.dram_tensor("ones16", [64, GPC], bf16, kind="ExternalInput")
    d_s1mask = nc.dram_tensor("s1mask", [80, 2, 64], bf16, kind="ExternalInput")
    d_out = nc.dram_tensor("out", [128, GPC], f32, kind="ExternalOutput")

    with tile.TileContext(nc) as tc:
        with (
            tc.tile_pool(name="wpool", bufs=1) as wp,      # resident weights
            tc.tile_pool(name="big", bufs=1) as bigp,      # resident activations
            tc.tile_pool(name="atp", bufs=6) as atp,       # streamed AT tiles
            tc.tile_pool(name="tmp", bufs=4) as tmp,       # small temporaries
            tc.tile_pool(name="psB", bufs=3, space="PSUM") as psB,
            tc.tile_pool(name="ps", bufs=4, space="PSUM") as psC,
            tc.tile_pool(name="psL", bufs=1, space="PSUM") as psLp,
        ):
            def load(dram, shape, eng):
                t = wp.tile(shape, dram.dtype, tag=dram.name)
                eng.dma_start(t[:], dram[:])
                return t

            # round-robin engine picker for relu/copy/cast work.
            # GPSIMD cannot read PSUM on this hw, so PSUM-sourced ops
            # alternate between vector (DVE) and scalar (Act) only.
            _rr = itertools.cycle([0, 1])

            def relu_to(out, in_):
                if next(_rr) == 0:
                    nc.vector.tensor_scalar_max(out, in_, 0.0)
                else:
                    nc.scalar.activation(out, in_, Relu)

            def copy_to(out, in_):
                if next(_rr) == 0:
                    nc.vector.tensor_copy(out, in_)
                else:
                    nc.scalar.copy(out, in_)

            # ---- resident tiles ----
            # f-dim padded to 336 with 100-wide chunks on 112-aligned offsets:
            # DoubleRow LDWEIGHTS requires 16B-aligned interleave stride/offset
            xf8 = wp.tile([128, 32, 2, 336], fp8, tag="xf8")
            Mxt = bigp.tile([100, 3, 1024], bf16, tag="Mxt")
            Gp0 = bigp.tile([128, 1024], bf16, tag="Gp0")
            Gp1 = bigp.tile([32, 1024], bf16, tag="Gp1")
            Ge0 = bigp.tile([64, 1024], bf16, tag="Ge0")
            Ge1 = bigp.tile([128, 1024], bf16, tag="Ge1")
            Ge2 = bigp.tile([108, 1024], bf16, tag="Ge2")
            H1p = bigp.tile([100, 3, 1024], bf16, tag="H1p")
            H1e = bigp.tile([120, 5, 1024], bf16, tag="H1e")
            Z = bigp.tile([128, 8, 300], bf16, tag="Z")
            S_bd = bigp.tile([128, 8, 160], bf16, tag="S_bd")
            ex0 = bigp.tile([128, 8, 10], f32, tag="ex0")
            sm0 = bigp.tile([128, 8], f32, tag="sm0")
            rc0 = bigp.tile([128, 8], f32, tag="rc0")
            X1T = bigp.tile([100, 3, 160], bf16, tag="X1T")
            T_bd = bigp.tile([128, 8, 160], bf16, tag="T_bd")
            T2_bd = bigp.tile([128, 8, 160], bf16, tag="T2_bd")
            A1bd = bigp.tile([80, 2, 160], bf16, tag="A1bd")
            A1Tbd = bigp.tile([80, 2, 160], bf16, tag="A1Tbd")
            Y1 = bigp.tile([80, 2, 450], bf16, tag="Y1")
            G1p = bigp.tile([75, 2, 160], bf16, tag="G1p")
            G1e = bigp.tile([100, 3, 160], bf16, tag="G1e")
            H1p1 = bigp.tile([100, 3, 160], bf16, tag="H1p1")
            H1e1 = bigp.tile([120, 5, 160], bf16, tag="H1e1")
            S1_bd = bigp.tile([80, 2, 64], bf16, tag="S1_bd")
            ex1 = bigp.tile([80, 2, 4], f32, tag="ex1")
            sm1 = bigp.tile([80, 2], f32, tag="sm1")
            rc1 = bigp.tile([80, 2], f32, tag="rc1")
            Z1 = bigp.tile([80, 2, 300], bf16, tag="Z1")
            X2T = bigp.tile([100, 3, 64], bf16, tag="X2T")
            T3 = bigp.tile([80, 2, 64], bf16, tag="T3")
            A2Tbd = bigp.tile([64, 64], bf16, tag="A2Tbd")
            Y2 = bigp.tile([64, 300], bf16, tag="Y2")
            G2 = bigp.tile([100, 3, 64], bf16, tag="G2")
            H2 = bigp.tile([120, 5, 64], bf16, tag="H2")
            Z2 = bigp.tile([64, 300], bf16, tag="Z2")
            X3T = bigp.tile([100, 3, GPC], bf16, tag="X3T")
            hT = bigp.tile([120, 5, GPC], bf16, tag="hT")

            # ---- prologue: zeros, first DMAs, act-table warm ----
            nc.gpsimd.memset(S_bd[:], 0.0)
            nc.gpsimd.memset(T_bd[:], 0.0)
            nc.gpsimd.memset(T2_bd[:], 0.0)
            dumm = tmp.tile([1, 2], f32, tag="dumm")
            nc.vector.memset(dumm[:], 0.0)
            nc.scalar.activation(dumm[:], dumm[:], Exp)  # preload exp table

            nc.sync.dma_start(xf8[:, 0:4, :, :], d_xf8[0])

            psBt = {0: [psB.tile([100, 512], f32, tag="psB", name=f"psB0_{c}")
                        for c in range(3)]}

            def b_iter(h, kk):
                at_t = atp.tile([128, 2, 512], fp8, tag="at")
                nc.sync.dma_start(at_t[:], d_at[h * 32 + kk])
                if h == 0 and kk % 4 == 0:
                    q = kk // 4 + 1
                    if q <= 7:
                        nc.sync.dma_start(xf8[:, 4*q:4*q+4, :, :], d_xf8[q])
                for c in range(3):
                    nc.tensor.matmul(
                        psBt[h][c][:], xf8[:, kk, :, c*112:c*112+100], at_t[:],
                        start=(kk == 0), stop=(kk == 31), perf_mode=DR)

            # ---- B'(h0) with weight loads woven in ----
            holder = {}
            for kk in range(32):
                b_iter(0, kk)
                if kk == 1:
                    holder["wcat0"] = load(d_wcat0, [100, 3, 492], nc.scalar)
                    holder["pWl0"] = load(d_pWl0, [128, 2, 300], nc.scalar)
                    holder["eWl0"] = load(d_eWl0, [128, 3, 600], nc.scalar)
                    holder["pWo0"] = load(d_pWo0, [100, 3, K0], nc.scalar)
                    holder["eWo0"] = load(d_eWo0, [120, 5, 300], nc.scalar)
                if kk == 10:
                    holder["wcat1"] = load(d_wcat1, [100, 3, 450], nc.scalar)
                    holder["pWl1"] = load(d_pWl1, [75, 2, 300], nc.scalar)
                    holder["pWo1"] = load(d_pWo1, [100, 3, K1], nc.scalar)
                    holder["eWl1"] = load(d_eWl1, [100, 3, 600], nc.scalar)
                    holder["eWo1"] = load(d_eWo1, [120, 5, 300], nc.scalar)
                    holder["adiag"] = load(d_adiag, [128, 8, 128], nc.gpsimd)
                    holder["adiagT"] = load(d_adiagT, [128, 8, 128], nc.gpsimd)
                if kk == 20:
                    holder["eWh2"] = load(d_eWh2, [100, 3, 300], nc.scalar)
                    holder["eWl2"] = load(d_eWl2, [100, 3, 600], nc.scalar)
                    holder["eWo2"] = load(d_eWo2, [120, 5, 300], nc.scalar)
                    holder["lW1"] = load(d_lW1, [100, 3, 600], nc.scalar)
                    holder["lW2"] = load(d_lW2, [120, 5, 128], nc.scalar)
                    holder["lb1"] = load(d_lb1, [120, 5], nc.gpsimd)
                    holder["lb2"] = load(d_lb2, [128, 1], nc.gpsimd)
                    holder["ones16"] = load(d_ones, [64, GPC], nc.gpsimd)
                    holder["s1mask"] = load(d_s1mask, [80, 2, 64], nc.gpsimd)
            wcat0 = holder["wcat0"]; pWl0 = holder["pWl0"]
            eWl0 = holder["eWl0"]; pWo0 = holder["pWo0"]; eWo0 = holder["eWo0"]
            wcat1 = holder["wcat1"]; pWl1 = holder["pWl1"]
            pWo1 = holder["pWo1"]; eWl1 = holder["eWl1"]; eWo1 = holder["eWo1"]
            adiag = holder["adiag"]; adiagT = holder["adiagT"]
            eWh2 = holder["eWh2"]; eWl2 = holder["eWl2"]; eWo2 = holder["eWo2"]
            lW1 = holder["lW1"]; lW2 = holder["lW2"]
            lb1 = holder["lb1"]; lb2 = holder["lb2"]
            ones16 = holder["ones16"]; s1mask = holder["s1mask"]

            def mxt_copies(h):
                sl = slice(h * 512, (h + 1) * 512)
                nc.vector.tensor_copy(Mxt[:, 0, sl], psBt[h][0][:])
                nc.scalar.copy(Mxt[:, 1, sl], psBt[h][1][:])
                nc.vector.tensor_copy(Mxt[:, 2, sl], psBt[h][2][:])

            # ---- chain stage emitters (all per half h) ----
            A_CHUNKS = [(0, 128), (128, 128), (256, 128), (384, 108)]

            def st_Ap(h, c2):
                off, sz = A_CHUNKS[c2]
                sl = slice(h * 512, (h + 1) * 512)
                ps = psC.tile([sz, 512], f32, tag="ps", name=f"psA{h}_{c2}")
                for kc in range(3):
                    nc.tensor.matmul(
                        ps[:], wcat0[:, kc, off:off + sz], Mxt[:, kc, sl],
                        start=(kc == 0), stop=(kc == 2))
                if c2 == 0:
                    relu_to(Gp0[:, sl], ps[:])
                elif c2 == 1:
                    relu_to(Gp1[:, sl], ps[0:32, :])
                    relu_to(Ge0[:, sl], ps[64:128, :])
                elif c2 == 2:
                    relu_to(Ge1[:, sl], ps[:])
                else:
                    relu_to(Ge2[:, sl], ps[0:108, :])

            def st_H1e(h, mc):
                sl = slice(h * 512, (h + 1) * 512)
                ps = psC.tile([120, 512], f32, tag="ps", name=f"psH1e{h}_{mc}")
                nc.tensor.matmul(ps[:], eWl0[0:64, 0, mc*120:(mc+1)*120],
                                 Ge0[:, sl], start=True, stop=False)
                nc.tensor.matmul(ps[:], eWl0[:, 1, mc*120:(mc+1)*120],
                                 Ge1[:, sl], start=False, stop=False)
                nc.tensor.matmul(ps[:], eWl0[0:108, 2, mc*120:(mc+1)*120],
                                 Ge2[:, sl], start=False, stop=True)
                relu_to(H1e[:, mc, sl], ps[:])

            def st_H1p(h, mc):
                sl = slice(h * 512, (h + 1) * 512)
                ps = psC.tile([100, 512], f32, tag="ps", name=f"psH1p{h}_{mc}")
                nc.tensor.matmul(ps[:], pWl0[:, 0, mc*100:(mc+1)*100],
                                 Gp0[:, sl], start=True, stop=False)
                nc.tensor.matmul(ps[:], pWl0[0:32, 1, mc*100:(mc+1)*100],
                                 Gp1[:, sl], start=False, stop=True)
                relu_to(H1p[:, mc, sl], ps[:])

            psL0 = {}

            def st_LZ(h, lm):
                m = 4 * h + lm
                if lm == 0:
                    psL0[h] = psLp.tile([128, 40], f32, tag="psL",
                                        name=f"psL0_{h}")
                for kc in range(3):
                    nc.tensor.matmul(
                        psL0[h][:, lm*10:(lm+1)*10],
                        H1p[:, kc, m*128:(m+1)*128], pWo0[:, kc, :],
                        start=(kc == 0), stop=(kc == 2))
                psz = psC.tile([128, 300], f32, tag="ps", name=f"psZ{m}")
                for kc in range(5):
                    nc.tensor.matmul(
                        psz[:], H1e[:, kc, m*128:(m+1)*128], eWo0[:, kc, :],
                        start=(kc == 0), stop=(kc == 4))
                copy_to(Z[:, m, :], psz[:])

            def st_smax0(h):
                exs = ex0[:, 4*h:4*h+4, :]
                nc.scalar.activation(
                    exs, psL0[h][:].rearrange("p (m k) -> p m k", k=10), Exp)
                nc.vector.reduce_sum(sm0[:, 4*h:4*h+4], exs, axis=AX)
                nc.vector.reciprocal(rc0[:, 4*h:4*h+4], sm0[:, 4*h:4*h+4])
                for lm in range(4):
                    m = 4 * h + lm
                    e = nc.vector if lm % 2 == 0 else nc.gpsimd
                    e.tensor_scalar_mul(
                        S_bd[0:64, m, m*20:m*20+10],
                        ex0[0:64, 4*h+lm, :], rc0[0:64, 4*h+lm:4*h+lm+1])
                    e.tensor_scalar_mul(
                        S_bd[64:128, m, m*20+10:m*20+20],
                        ex0[64:128, 4*h+lm, :], rc0[64:128, 4*h+lm:4*h+lm+1])

            def st_X1T(h, mc):
                ps = psC.tile([100, 80], f32, tag="ps", name=f"psX1T{h}_{mc}")
                for k in range(4 * h, 4 * h + 4):
                    nc.tensor.matmul(
                        ps[:], Z[:, k, mc*100:(mc+1)*100],
                        S_bd[:, k, h*80:h*80+80],
                        start=(k == 4 * h), stop=(k == 4 * h + 3))
                copy_to(X1T[:, mc, h*80:h*80+80], ps[:])

            def st_T(h, c):
                cs = slice(c * 20, c * 20 + 20)
                psT = psC.tile([128, 20], f32, tag="ps", name=f"psT{c}")
                nc.tensor.matmul(psT[:], adiagT[:, c, :], S_bd[:, c, cs],
                                 start=True, stop=True)
                psT2 = psC.tile([128, 20], f32, tag="ps", name=f"psT2{c}")
                nc.tensor.matmul(psT2[:], adiag[:, c, :], S_bd[:, c, cs],
                                 start=True, stop=True)
                copy_to(T_bd[:, c, cs], psT[:])
                copy_to(T2_bd[:, c, cs], psT2[:])

            def st_A1(h, variant):
                src = T_bd if variant == 0 else T2_bd
                dst = A1bd if variant == 0 else A1Tbd
                hs = slice(h * 80, h * 80 + 80)
                ps = psC.tile([80, 80], f32, tag="ps", name=f"psA1{h}_{variant}")
                for k in range(4 * h, 4 * h + 4):
                    nc.tensor.matmul(
                        ps[:], S_bd[:, k, hs], src[:, k, hs],
                        start=(k == 4 * h), stop=(k == 4 * h + 3))
                copy_to(dst[:, h, hs], ps[:])

            def chain_stages(h):
                st = []
                for c2 in range(4):
                    st.append(lambda c2=c2: st_Ap(h, c2))
                for mc in range(5):
                    st.append(lambda mc=mc: st_H1e(h, mc))
                for mc in range(3):
                    st.append(lambda mc=mc: st_H1p(h, mc))
                for lm in range(4):
                    st.append(lambda lm=lm: st_LZ(h, lm))
                st.append(lambda: st_smax0(h))
                for mc in range(3):
                    st.append(lambda mc=mc: st_X1T(h, mc))
                for c in range(4 * h, 4 * h + 4):
                    st.append(lambda c=c: st_T(h, c))
                st.append(lambda: st_A1(h, 0))
                st.append(lambda: st_A1(h, 1))
                return st

            # ---- level 1 stages ----
            def st_Y1(mi):
                ps = psC.tile([80, 450], f32, tag="ps", name=f"psY1_{mi}")
                for kc in range(3):
                    nc.tensor.matmul(
                        ps[:], X1T[:, kc, mi*80:(mi+1)*80], wcat1[:, kc, :],
                        start=(kc == 0), stop=(kc == 2))
                copy_to(Y1[:, mi, :], ps[:])

            def st_G1p(h):
                hs = slice(h * 80, h * 80 + 80)
                for fc in range(2):
                    ps = psC.tile([75, 80], f32, tag="ps", name=f"psG1p{h}_{fc}")
                    nc.tensor.matmul(ps[:], Y1[:, h, fc*75:(fc+1)*75],
                                     A1Tbd[:, h, hs], start=True, stop=True)
                    relu_to(G1p[:, fc, hs], ps[:])

            def st_G1e(h):
                hs = slice(h * 80, h * 80 + 80)
                for fc in range(3):
                    ps = psC.tile([100, 80], f32, tag="ps", name=f"psG1e{h}_{fc}")
                    nc.tensor.matmul(ps[:], Y1[:, h, 150+fc*100:150+(fc+1)*100],
                                     A1Tbd[:, h, hs], start=True, stop=True)
                    relu_to(G1e[:, fc, hs], ps[:])

            def st_H1p1(h, mc):
                hs = slice(h * 80, h * 80 + 80)
                ps = psC.tile([100, 80], f32, tag="ps", name=f"psH1p1{h}_{mc}")
                for kc in range(2):
                    nc.tensor.matmul(
                        ps[:], pWl1[:, kc, mc*100:(mc+1)*100], G1p[:, kc, hs],
                        start=(kc == 0), stop=(kc == 1))
                relu_to(H1p1[:, mc, hs], ps[:])

            def st_H1e1(h, mc):
                hs = slice(h * 80, h * 80 + 80)
                ps = psC.tile([120, 80], f32, tag="ps", name=f"psH1e1{h}_{mc}")
                for kc in range(3):
                    nc.tensor.matmul(
                        ps[:], eWl1[:, kc, mc*120:(mc+1)*120], G1e[:, kc, hs],
                        start=(kc == 0), stop=(kc == 2))
                relu_to(H1e1[:, mc, hs], ps[:])

            def st_S1(mi):
                psl = psLp.tile([80, K1], f32, tag="psL", name=f"psL1_{mi}")
                for kc in range(3):
                    nc.tensor.matmul(
                        psl[:], H1p1[:, kc, mi*80:(mi+1)*80], pWo1[:, kc, :],
                        start=(kc == 0), stop=(kc == 2))
                nc.scalar.activation(ex1[:, mi, :], psl[:], Exp)
                nc.vector.reduce_sum(sm1[:, mi:mi+1], ex1[:, mi, :], axis=AX)
                nc.vector.reciprocal(rc1[:, mi:mi+1], sm1[:, mi:mi+1])
                nc.vector.scalar_tensor_tensor(
                    S1_bd[:, mi, :].rearrange("p (b j) -> p b j", j=K1),
                    ex1[:, mi:mi+1, :].to_broadcast((80, GPC, K1)),
                    rc1[:, mi:mi+1],
                    s1mask[:, mi, :].rearrange("p (b j) -> p b j", j=K1),
                    Mul, Mul)

            def st_Z1(mi):
                ps = psC.tile([80, 300], f32, tag="ps", name=f"psZ1_{mi}")
                for kc in range(5):
                    nc.tensor.matmul(
                        ps[:], H1e1[:, kc, mi*80:(mi+1)*80], eWo1[:, kc, :],
                        start=(kc == 0), stop=(kc == 4))
                copy_to(Z1[:, mi, :], ps[:])

            def st_X2T(h):
                gs = slice(h * 32, h * 32 + 32)
                for mc in range(3):
                    ps = psC.tile([100, 32], f32, tag="ps", name=f"psX2T{h}_{mc}")
                    nc.tensor.matmul(ps[:], Z1[:, h, mc*100:(mc+1)*100],
                                     S1_bd[:, h, gs], start=True, stop=True)
                    copy_to(X2T[:, mc, gs], ps[:])

            def st_TA2(h):
                gs = slice(h * 32, h * 32 + 32)
                hs = slice(h * 80, h * 80 + 80)
                ps = psC.tile([80, 64], f32, tag="ps", name=f"psT3_{h}")
                nc.tensor.matmul(ps[:], A1bd[:, h, hs], S1_bd[:, h, :],
                                 start=True, stop=True)
                copy_to(T3[:, h, :], ps[:])
                ps2 = psC.tile([32, 32], f32, tag="ps", name=f"psA2_{h}")
                nc.tensor.matmul(ps2[:], S1_bd[:, h, gs], T3[:, h, gs],
                                 start=True, stop=True)
                copy_to(A2Tbd[h*32:h*32+32, gs], ps2[:])

            # ---- level 2 stages ----
            def st_Y2(h):
                gs = slice(h * 32, h * 32 + 32)
                ps = psC.tile([32, 300], f32, tag="ps", name=f"psY2_{h}")
                for kc in range(3):
                    nc.tensor.matmul(ps[:], X2T[:, kc, gs], eWh2[:, kc, :],
                                     start=(kc == 0), stop=(kc == 2))
                copy_to(Y2[gs, :], ps[:])

            def st_G2(h):
                gs = slice(h * 32, h * 32 + 32)
                for mf in range(3):
                    ps = psC.tile([100, 32], f32, tag="ps", name=f"psG2{h}_{mf}")
                    nc.tensor.matmul(ps[:], Y2[gs, mf*100:(mf+1)*100],
                                     A2Tbd[gs, gs], start=True, stop=True)
                    relu_to(G2[:, mf, gs], ps[:])

            def st_H2(h, mcs):
                gs = slice(h * 32, h * 32 + 32)
                for mc in mcs:
                    ps = psC.tile([120, 32], f32, tag="ps", name=f"psH2{h}_{mc}")
                    for kc in range(3):
                        nc.tensor.matmul(
                            ps[:], eWl2[:, kc, mc*120:(mc+1)*120],
                            G2[:, kc, gs], start=(kc == 0), stop=(kc == 2))
                    relu_to(H2[:, mc, gs], ps[:])

            def st_Z2(h):
                gs = slice(h * 32, h * 32 + 32)
                ps = psC.tile([32, 300], f32, tag="ps", name=f"psZ2_{h}")
                for kc in range(5):
                    nc.tensor.matmul(ps[:], H2[:, kc, gs], eWo2[:, kc, :],
                                     start=(kc == 0), stop=(kc == 4))
                copy_to(Z2[gs, :], ps[:])

            def st_X3T(h):
                gs = slice(h * 32, h * 32 + 32)
                for mf in range(3):
                    ps = psC.tile([100, 8], f32, tag="ps", name=f"psX3{h}_{mf}")
                    nc.tensor.matmul(ps[:], Z2[gs, mf*100:(mf+1)*100],
                                     ones16[gs, h*8:h*8+8],
                                     start=True, stop=True)
                    copy_to(X3T[:, mf, h*8:h*8+8], ps[:])

            def st_hT(h, mcs):
                for mc in mcs:
                    ps = psC.tile([120, 8], f32, tag="ps", name=f"psh{h}_{mc}")
                    for kc in range(3):
                        nc.tensor.matmul(
                            ps[:], lW1[:, kc, mc*120:(mc+1)*120],
                            X3T[:, kc, h*8:h*8+8],
                            start=(kc == 0), stop=(kc == 2))
                    nc.scalar.activation(hT[:, mc, h*8:h*8+8], ps[:], Relu,
                                         bias=lb1[:, mc:mc+1])

            # ================= emission schedule =================
            mxt_copies(0)
            # allocate h1 accumulators only now: the pool-cycling WAR dep
            # must land on the h0->Mxt copies just emitted
            psBt[1] = [psB.tile([100, 512], f32, tag="psB", name=f"psB1_{c}")
                       for c in range(3)]

            # region 1: chain(h0) woven with B'(h1)
            b1 = iter(range(32))

            def emit_b(n):
                for _ in range(n):
                    kk = next(b1, None)
                    if kk is not None:
                        b_iter(1, kk)

            budget = [2, 1, 1, 1,          # A'
                      1, 1, 1, 1, 1,       # H1e
                      1, 1, 1,             # H1p
                      2, 2, 2, 2,          # LZ
                      2,                   # smax
                      1, 1, 1,             # X1T
                      1, 1, 1, 1,          # T
                      1, 1]                # A1
            for nb, st in zip(budget, chain_stages(0)):
                emit_b(nb)
                st()
            emit_b(32)  # flush any remainder
            mxt_copies(1)

            # region 2: chain(h1) woven with level-1(h0)
            l1h0 = ([lambda: st_Y1(0), lambda: st_G1p(0), lambda: st_G1e(0)]
                    + [lambda mc=mc: st_H1p1(0, mc) for mc in range(3)]
                    + [lambda mc=mc: st_H1e1(0, mc) for mc in range(5)])
            ch1 = chain_stages(1)
            l1i = iter(l1h0)
            for i, st in enumerate(ch1):
                st()
                if i >= 4 and i % 2 == 0:
                    nxt = next(l1i, None)
                    if nxt is not None:
                        nxt()
            for nxt in l1i:
                nxt()

            # region 3: rest of level-1(h0) + level-1(h1) + level-2(h0)
            st_S1(0)
            st_Y1(1)
            st_Z1(0)
            st_G1p(1)
            st_X2T(0)
            st_G1e(1)
            st_TA2(0)
            st_H1p1(1, 0)
            st_Y2(0)
            st_H1p1(1, 1)
            st_H1p1(1, 2)
            st_G2(0)
            st_H1e1(1, 0)
            st_H1e1(1, 1)
            st_H2(0, [0, 1, 2])
            st_H1e1(1, 2)
            st_H1e1(1, 3)
            st_H2(0, [3, 4])
            st_H1e1(1, 4)
            st_Z2(0)
            st_S1(1)
            st_X3T(0)
            st_Z1(1)
            st_hT(0, [0, 1])
            st_X2T(1)
            st_hT(0, [2, 3, 4])
            st_TA2(1)

            # region 4: level-2(h1) + head
            st_Y2(1)
            st_G2(1)
            st_H2(1, [0, 1, 2])
            st_H2(1, [3, 4])
            st_Z2(1)
            st_X3T(1)
            st_hT(1, [0, 1, 2])
            st_hT(1, [3, 4])

            psO = psC.tile([128, GPC], f32, tag="ps", name="psO")
            for kc in range(5):
                nc.tensor.matmul(
                    psO[:], lW2[:, kc, :], hT[:, kc, :],
                    start=(kc == 0), stop=(kc == 4))
            outT = tmp.tile([128, GPC], f32, tag="outT")
            nc.vector.tensor_scalar_add(outT[:], psO[:], lb2[:])
            nc.sync.dma_start(d_out[:], outT[:])

    _split_excess_waits(nc)
    return nc


def _host_prep(inputs):
    """Build per-core in_maps from the full inputs."""
    ONE = np.uint8(0x38)  # 1.0 in float8_e4m3

    x = np.asarray(inputs["x"], np.float32)
    ei = np.asarray(inputs["edge_index"]).astype(np.int64)

    # full A^T in fp8 bit pattern: AT[j, i] = A[i, j]
    ATu = np.zeros((N_NODES, N_NODES), np.uint8)
    ATu[ei[1], ei[0]] = ONE

    # x in fp8, j index split as j = kk*256 + r*128 + p, chunks of 4 kk.
    # f padded 300 -> 336: 100-wide chunks at 112-aligned offsets (DoubleRow
    # LDWEIGHTS needs 16B-aligned interleave stride and start offset).
    xpad = np.zeros((N_NODES, 336), np.float32)
    for c in range(3):
        xpad[:, 112 * c:112 * c + 100] = x[:, 100 * c:100 * c + 100]
    xf8 = np.ascontiguousarray(
        xpad.reshape(8, 4, 2, 128, 336).transpose(0, 3, 1, 2, 4)).astype(F8)

    def chunkw(w, p, c):
        w = np.asarray(w, np.float32)
        return np.ascontiguousarray(
            w.reshape(c, p, -1).transpose(1, 0, 2)).astype(BF)

    def padchunk(w, rowchunks, c, m):
        w = np.asarray(w, np.float32)
        out = np.zeros((128, c, m), np.float32)
        for ci, (a, b) in enumerate(rowchunks):
            out[0:b - a, ci, :] = w[a:b, :]
        return out.astype(BF)

    wcat0 = np.zeros((300, 492), np.float32)
    wcat0[:, 0:150] = np.asarray(inputs["pWh0"], np.float32)
    wcat0[:, 192:492] = np.asarray(inputs["eWh0"], np.float32)
    wcat1 = np.zeros((300, 450), np.float32)
    wcat1[:, 0:150] = np.asarray(inputs["pWh1"], np.float32)
    wcat1[:, 150:450] = np.asarray(inputs["eWh1"], np.float32)

    ones16 = np.zeros((64, GPC), BF)
    for b in range(GPC):
        ones16[b * 4:(b + 1) * 4, b] = 1
    s1mask = np.zeros((80, 2, 64), BF)
    for mi in range(2):
        for p in range(80):
            gb = (80 * mi + p) // K0
            s1mask[p, mi, gb * 4:(gb + 1) * 4] = 1
    lb1 = np.ascontiguousarray(
        np.asarray(inputs["lb1"], np.float32).reshape(5, 120).T)
    lb2 = np.asarray(inputs["lb2"], np.float32).reshape(128, 1)

    shared = {
        "xf8": xf8,
        "wcat0": chunkw(wcat0, 100, 3),
        "pWl0": padchunk(inputs["pWl0"], [(0, 128), (128, 150)], 2, 300),
        "pWo0": chunkw(inputs["pWo0"], 100, 3),
        "eWl0": padchunk(inputs["eWl0"], [(0, 64), (64, 192), (192, 300)], 3, 600),
        "eWo0": chunkw(inputs["eWo0"], 120, 5),
        "wcat1": chunkw(wcat1, 100, 3),
        "pWl1": chunkw(inputs["pWl1"], 75, 2),
        "pWo1": chunkw(inputs["pWo1"], 100, 3),
        "eWl1": chunkw(inputs["eWl1"], 100, 3),
        "eWo1": chunkw(inputs["eWo1"], 120, 5),
        "eWh2": chunkw(inputs["eWh2"], 100, 3),
        "eWl2": chunkw(inputs["eWl2"], 100, 3),
        "eWo2": chunkw(inputs["eWo2"], 120, 5),
        "lW1": chunkw(inputs["lW1"], 100, 3),
        "lW2": chunkw(inputs["lW2"], 120, 5),
        "lb1": lb1,
        "lb2": lb2,
        "ones16": ones16,
        "s1mask": s1mask,
    }

    in_maps = []
    for d in range(N_CORES):
        r0 = d * R
        slab = ATu[:, r0:r0 + R]  # [8192, 1024]
        at = np.concatenate(
            [np.ascontiguousarray(
                slab[:, 512*h:512*(h+1)].reshape(32, 2, 128, 512)
                .transpose(0, 2, 1, 3))
             for h in range(2)], axis=0).view(F8)

        adiag = np.zeros((128, 8, 128), np.uint8)
        adiagT = np.zeros((128, 8, 128), np.uint8)
        for c in range(8):
            # full 128x128 slab block, then mask to per-graph 64x64 diag
            blkT = slab[r0 + 128 * c: r0 + 128 * (c + 1),
                        128 * c: 128 * (c + 1)]  # blkT[q, p] = A[rows p, cols q]
            blk = blkT.T
            for hh in range(2):
                s = slice(64 * hh, 64 * (hh + 1))
                adiag[s, c, s] = blk[s, s]
                adiagT[s, c, s] = blkT[s, s]
        m = dict(shared)
        m["at"] = at
        m["adiag"] = adiag.view(F8).astype(BF)
        m["adiagT"] = adiagT.view(F8).astype(BF)
        in_maps.append(m)
    return in_maps


def _run(inputs, trace=False, trace_kwargs=None):
    try:
        import concourse.bass as bass  # noqa: F401
    except ImportError:
        import sys
        sys.path.insert(0, "/opt/trn_rl_repo")
    from concourse.bass_utils import run_bass_kernel_spmd

    if "prog" not in _prog_cache:
        _prog_cache["prog"] = _build_program()
    nc = _prog_cache["prog"]

    in_maps = _host_prep(inputs)
    res = run_bass_kernel_spmd(
        nc, in_maps, core_ids=list(range(N_CORES)), trace=trace,
        **(trace_kwargs or {}),
    )
    out = np.empty((B, 128), np.float32)
    for d in range(N_CORES):
        out[d * GPC:(d + 1) * GPC, :] = res.results[d]["out"].T
    return out, res


def kernel(**inputs):
    out, _ = _run(inputs, trace=False)
    return out
